# revision 7
# baseline (speedup 1.0000x reference)
"""Trainium2 Bass kernel for nn_Decoder_24541443129406.

Math: the reference's pdf/pdf_max cancels the normalization, so

    prob[n] = clip( sum_m exp( -0.5 * sum_d (pos[n,d]-mean[m,d])^2 / sigma[m,d] ), 0, 1 )

with pos = [ox, oy, dx, dy], sigma = [sx, sy, 1e-3, 1e-3],
sx = relu(l4)+0.01, sy = relu(l5)+0.01, mean = latents[:, :4].

The exponent is a quadratic form -> a K=8 matmul:
    e[n,m] = f[n] . w[m]
    f[n] = [dx^2+dy^2, 1, ox, oy, dx, dy, ox^2, oy^2]
    w[m] = [c7, c0, c1, c2, c3, c4, c5, c6]
      c1 = mx/sx, c2 = my/sy, c3 = 1000*mdx, c4 = 1000*mdy,
      c5 = -0.5/sx, c6 = -0.5/sy, c7 = -500,
      c0 = -0.5*(mx^2/sx + my^2/sy + 1000*(mdx^2+mdy^2))

Per core (8 cores, data-parallel over rays): N_loc = 8192 rays, M = 512
gaussians. 64 ray-blocks of 128: PE matmul lhsT=[8,128] rhs=[8,512] ->
PSUM [128 rays, 512 gauss]; ACT Exp with accum_out -> per-ray sums;
clip; scatter-DMA out.
"""

import os
import sys
from contextlib import ExitStack

import numpy as np

for _p in ("/opt/trn_rl_repo", "/root/.axon_site/_ro/trn_rl_repo"):
    if os.path.isdir(_p) and _p not in sys.path:
        sys.path.insert(0, _p)

import concourse.bacc as bacc
import concourse.bass as bass
import concourse.mybir as mybir
import concourse.tile as tile
from concourse import bass_utils

N_CORES = 8
N = 65536
M = 512
N_LOC = N // N_CORES  # 8192
NCHUNK = 32  # feature-build chunks (32-partition groups: verifier requires
# compute-op SBUF APs to start at partition 0/32/64/96)
CHUNK = N_LOC // NCHUNK  # 256
NBLK = N_LOC // 128  # 64 ray blocks of 128
SIGMA_EPS = 0.01
INV_SDIR = 1000.0  # 1/sigma_dir

F32 = mybir.dt.float32
ALU = mybir.AluOpType
ACTF = mybir.ActivationFunctionType

TRACE = False
LAST_PERF = None
_CACHED_NC = None


def build_kernel_body(nc, origins, directions, latents, prob):
    """origins/directions: [N_LOC, 2] f32 DRAM APs; latents [M, 6]; prob [N_LOC, 1]."""
    with tile.TileContext(nc) as tc, ExitStack() as ctx:
        singles = ctx.enter_context(tc.tile_pool(name="singles", bufs=1))
        psum = ctx.enter_context(tc.tile_pool(name="psum", bufs=6, space="PSUM"))
        scratch = ctx.enter_context(tc.tile_pool(name="scratch", bufs=3))

        # ---------------- gaussian weights wg [8, M] ----------------
        # wg rows: 0=c7, 1=c0, 2=c1, 3=c2, 4=c3, 5=c4, 6=c5, 7=c6
        lat_s = singles.tile([2, M], F32)  # lx, ly
        lat_m = singles.tile([2, M], F32)  # mx, my
        lat_d = singles.tile([2, M], F32)  # mdx, mdy
        lat_t = latents.rearrange("m f -> f m")  # [6, M] strided view of DRAM
        nc.sync.dma_start(out=lat_s, in_=lat_t[4:6, :])
        nc.sync.dma_start(out=lat_m, in_=lat_t[0:2, :])
        nc.sync.dma_start(out=lat_d, in_=lat_t[2:4, :])

        # s = relu(l)+eps ; r = 1/s   (all ops partition-aligned: rows 0-1)
        nc.vector.tensor_scalar(
            out=lat_s, in0=lat_s, scalar1=0.0, scalar2=SIGMA_EPS,
            op0=ALU.max, op1=ALU.add,
        )
        rec = singles.tile([2, M], F32)
        nc.vector.reciprocal(out=rec, in_=lat_s)

        c12 = singles.tile([2, M], F32)  # mx/sx, my/sy
        nc.vector.tensor_mul(out=c12, in0=lat_m, in1=rec)
        c34 = singles.tile([2, M], F32)  # 1000*mdx, 1000*mdy
        nc.vector.tensor_scalar_mul(out=c34, in0=lat_d, scalar1=INV_SDIR)
        c56 = singles.tile([2, M], F32)  # -0.5/sx, -0.5/sy
        nc.vector.tensor_scalar_mul(out=c56, in0=rec, scalar1=-0.5)

        # c0 = -0.5*(mx*c1 + my*c2 + mdx*c3 + mdy*c4)
        p1 = singles.tile([2, M], F32)
        nc.vector.tensor_mul(out=p1, in0=lat_m, in1=c12)
        p2 = singles.tile([2, M], F32)
        nc.vector.tensor_mul(out=p2, in0=lat_d, in1=c34)
        nc.vector.tensor_add(out=p1, in0=p1, in1=p2)
        # cross-partition: move row 1 next to row 0, then aligned add
        p1b = singles.tile([1, M], F32)
        nc.sync.dma_start(out=p1b, in_=p1[1:2, :])
        nc.vector.tensor_add(out=p1[0:1, :], in0=p1[0:1, :], in1=p1b)

        wg = singles.tile([8, M], F32)
        nc.vector.memset(wg[0:1, :], -0.5 * INV_SDIR)  # c7
        nc.vector.tensor_scalar_mul(out=p1[0:1, :], in0=p1[0:1, :], scalar1=-0.5)  # c0
        # assemble (cross-partition moves via DMA)
        nc.sync.dma_start(out=wg[1:2, :], in_=p1[0:1, :])
        nc.sync.dma_start(out=wg[2:4, :], in_=c12)
        nc.sync.dma_start(out=wg[4:6, :], in_=c34)
        nc.sync.dma_start(out=wg[6:8, :], in_=c56)

        # ---------------- feature tile build ----------------
        # Two f-major tiles, 32-partition feature groups (verifier-legal starts).
        # featA groups: 0=dx^2+dy^2, 1=ones, 2=ox, 3=oy
        # featB groups: 4=dx, 5=dy, 6=ox^2, 7=oy^2
        featA = singles.tile([128, CHUNK], F32)
        featB = singles.tile([128, CHUNK], F32)
        s1 = singles.tile([NCHUNK, CHUNK], F32)

        og = origins.rearrange("(i r) c -> i r c", i=NCHUNK)  # [32, 256, 2]
        dr = directions.rearrange("(i r) c -> i r c", i=NCHUNK)

        nc.sync.dma_start(out=featA[0:32, :], in_=dr[:, :, 0])    # dx (-> dx^2+dy^2)
        nc.vector.memset(featA[32:64, :], 1.0)                    # ones
        nc.sync.dma_start(out=featA[64:96, :], in_=og[:, :, 0])   # ox
        nc.sync.dma_start(out=featA[96:128, :], in_=og[:, :, 1])  # oy
        nc.sync.dma_start(out=featB[0:32, :], in_=dr[:, :, 0])    # dx
        nc.sync.dma_start(out=featB[32:64, :], in_=dr[:, :, 1])   # dy
        nc.sync.dma_start(out=featB[64:96, :], in_=og[:, :, 0])   # ox (-> ox^2)
        nc.sync.dma_start(out=featB[96:128, :], in_=og[:, :, 1])  # oy (-> oy^2)
        nc.sync.dma_start(out=s1, in_=dr[:, :, 1])                # dy (-> dy^2)

        nc.vector.tensor_mul(out=featA[0:32, :], in0=featA[0:32, :], in1=featA[0:32, :])
        nc.vector.tensor_mul(out=s1, in0=s1, in1=s1)
        nc.vector.tensor_add(out=featA[0:32, :], in0=featA[0:32, :], in1=s1)
        nc.vector.tensor_mul(out=featB[64:96, :], in0=featB[64:96, :], in1=featB[64:96, :])
        nc.vector.tensor_mul(out=featB[96:128, :], in0=featB[96:128, :], in1=featB[96:128, :])

        # permute to featc [8, N_LOC]: partition = f, col = i*CHUNK + r.
        # One DMA per feature group: [32, CHUNK] partition-major stream ==
        # C-order [1, N_LOC] row.
        featc = singles.tile([8, N_LOC], F32)
        for f in range(4):
            nc.sync.dma_start(
                out=featc[f : f + 1, :], in_=featA[32 * f : 32 * (f + 1), :]
            )
            nc.sync.dma_start(
                out=featc[4 + f : 5 + f, :], in_=featB[32 * f : 32 * (f + 1), :]
            )

        # ---------------- main loop ----------------
        res = singles.tile([128, NBLK], F32)  # res[p, b] = sum_m exp(e) for ray 128b+p
        for b in range(NBLK):
            ps = psum.tile([128, M], F32, tag="ps")
            nc.tensor.matmul(
                out=ps,
                lhsT=featc[:, 128 * b : 128 * (b + 1)],
                rhs=wg,
                start=True,
                stop=True,
            )
            ex = scratch.tile([128, M], F32, tag="ex")
            nc.scalar.activation(
                out=ex,
                in_=ps,
                func=ACTF.Exp,
                accum_out=res[:, b : b + 1],
            )

        # clip to [0, 1]
        nc.vector.tensor_scalar(
            out=res, in0=res, scalar1=0.0, scalar2=1.0, op0=ALU.max, op1=ALU.min
        )

        # scatter out: prob[128b + p] = res[p, b]; split across 8 DMAs (queues)
        pv = prob.rearrange("(b p) o -> p (b o)", p=128)  # [128, NBLK] view
        for q in range(8):
            nc.sync.dma_start(
                out=pv[16 * q : 16 * (q + 1), :], in_=res[16 * q : 16 * (q + 1), :]
            )


def build_nc():
    nc = bacc.Bacc("TRN2", target_bir_lowering=False, debug=False)
    origins = nc.dram_tensor("origins", [N_LOC, 2], F32, kind="ExternalInput").ap()
    directions = nc.dram_tensor("directions", [N_LOC, 2], F32, kind="ExternalInput").ap()
    latents = nc.dram_tensor("latents", [M, 6], F32, kind="ExternalInput").ap()
    prob = nc.dram_tensor("prob", [N_LOC, 1], F32, kind="ExternalOutput").ap()
    build_kernel_body(nc, origins, directions, latents, prob)
    nc.compile()
    return nc


def kernel(origins: np.ndarray, directions: np.ndarray, latents: np.ndarray) -> np.ndarray:
    global _CACHED_NC, LAST_PERF
    assert origins.shape == (N, 2) and directions.shape == (N, 2)
    assert latents.shape == (M, 6)
    origins = np.ascontiguousarray(origins, dtype=np.float32)
    directions = np.ascontiguousarray(directions, dtype=np.float32)
    latents = np.ascontiguousarray(latents, dtype=np.float32)

    if _CACHED_NC is None:
        _CACHED_NC = build_nc()
    nc = _CACHED_NC

    in_maps = []
    for c in range(N_CORES):
        sl = slice(c * N_LOC, (c + 1) * N_LOC)
        in_maps.append(
            {
                "origins": origins[sl],
                "directions": directions[sl],
                "latents": latents,
            }
        )

    results = bass_utils.run_bass_kernel_spmd(
        nc,
        in_maps,
        core_ids=list(range(N_CORES)),
        trace=TRACE,
    )
    LAST_PERF = results
    out = np.concatenate([results.results[c]["prob"] for c in range(N_CORES)], axis=0)
    return out.astype(np.float32)


if __name__ == "__main__":
    rng = np.random.default_rng(0)
    o = rng.standard_normal((N, 2), dtype=np.float32)
    d = rng.standard_normal((N, 2), dtype=np.float32)
    l = rng.standard_normal((M, 6), dtype=np.float32)
    p = kernel(o, d, l)
    print(p.shape, p.dtype, p.min(), p.max())
